# revision 1
# baseline (speedup 1.0000x reference)
"""Trainium2 Bass kernel for nn_AttachmentPredictor.

Computation (per batch row b):
  head = x[b, :-2, :] @ proj_head + x[b,-2,:] @ proj_prep + x[b,-1,:] @ proj_child
  composed = tanh(head)                      # [T-2, P]
  composed = tanh(composed @ hidden_W[0])
  composed = tanh(composed @ hidden_W[1])
  scores = composed @ scorer                 # [T-2]
  out = where(mask, exp(scores), 0); out /= (sum(out) + 1e-7)

Sharding: pure data parallel, batch 64 -> 8 rows per core on 8 cores.

On-chip layout: activations kept transposed [P on partitions, tokens on free
dim].  x tiles are loaded naturally [tok, d] and transposed on the tensor
engine ([128,128] blocks via identity matmul).  All matmuls use float32r
(full-rate fp32 streaming).  The 2046 head tokens per row are processed as
2048 (the prep/child rows ride along as garbage and are masked out).
"""

import sys

import numpy as np

sys.path.insert(0, "/opt/trn_rl_repo")

B = 64
T = 2048
TH = 2046  # head tokens
D = 1024
P = 512
NCORES = 8
R = B // NCORES  # 8 batch rows per core
KD = D // 128  # 8 contraction chunks for layer 1
KP = P // 128  # 4 contraction chunks for layers 2/3/scorer
NTOK = 512  # tokens per chunk
CH = T // NTOK  # 4 chunks per row
J16 = T // 128  # 16 score sub-chunks of 128 tokens per row

X_BF16 = False  # if True: ship x and layer-1 weights as bf16
SAFE_SCORER = False  # if True: scorer matmuls write bank-offset-0 PSUM tiles
_CACHE = {}


def _build(R=R):
    import concourse.bass as bass
    import concourse.mybir as mybir
    import concourse.tile as tile
    from concourse import bacc
    from concourse.masks import make_identity

    f32 = mybir.dt.float32
    f32r = mybir.dt.float32r
    bf16 = mybir.dt.bfloat16
    u8 = mybir.dt.uint8
    xdt = bf16 if X_BF16 else f32r
    bdt = bf16 if X_BF16 else f32
    AF = mybir.ActivationFunctionType
    ALU = mybir.AluOpType

    nc = bacc.Bacc(
        "TRN2", target_bir_lowering=False, debug=False, num_devices=NCORES
    )

    xs = nc.dram_tensor("xs", [R, T, D], xdt, kind="ExternalInput").ap()
    w1 = nc.dram_tensor("w1", [D, P], xdt, kind="ExternalInput").ap()
    wp = nc.dram_tensor("wp", [D, P], bdt, kind="ExternalInput").ap()
    wc = nc.dram_tensor("wc", [D, P], bdt, kind="ExternalInput").ap()
    h0 = nc.dram_tensor("h0", [P, P], f32r, kind="ExternalInput").ap()
    h1 = nc.dram_tensor("h1", [P, P], f32r, kind="ExternalInput").ap()
    sc = nc.dram_tensor("sc", [P, 1], f32, kind="ExternalInput").ap()
    mk = nc.dram_tensor("mk", [R, T], u8, kind="ExternalInput").ap()
    out = nc.dram_tensor("out", [R, TH], f32, kind="ExternalOutput").ap()

    with tile.TileContext(nc) as tc:
        with (
            tc.tile_pool(name="wpool", bufs=1) as wpool,
            tc.tile_pool(name="cpool", bufs=1) as cpool,
            tc.tile_pool(name="xn_pool", bufs=2) as xn_pool,
            tc.tile_pool(name="xt_pool", bufs=2 * KD) as xt_pool,
            tc.tile_pool(name="y_pool", bufs=2 * KP) as y_pool,
            tc.tile_pool(name="tail_pool", bufs=2) as tail_pool,
            tc.tile_pool(name="xtp_pool", bufs=2, space="PSUM") as xtp_pool,
            tc.tile_pool(name="mmp_pool", bufs=3, space="PSUM") as mmp_pool,
            tc.tile_pool(name="scp_pool", bufs=1, space="PSUM") as scp_pool,
            tc.tile_pool(name="tlp_pool", bufs=2, space="PSUM") as tlp_pool,
        ):
            # ---- weights: [p, k, q] = W[k*128 + p, q] ----
            w1t = wpool.tile([128, KD, P], xdt)
            wpt = wpool.tile([128, KD, P], bdt)
            wct = wpool.tile([128, KD, P], bdt)
            for k in range(KD):
                nc.sync.dma_start(w1t[:, k, :], w1[k * 128 : (k + 1) * 128, :])
                nc.sync.dma_start(wpt[:, k, :], wp[k * 128 : (k + 1) * 128, :])
                nc.sync.dma_start(wct[:, k, :], wc[k * 128 : (k + 1) * 128, :])
            h0t = wpool.tile([128, KP, P], f32r)
            h1t = wpool.tile([128, KP, P], f32r)
            sct = wpool.tile([128, KP], f32)
            for k in range(KP):
                nc.sync.dma_start(h0t[:, k, :], h0[k * 128 : (k + 1) * 128, :])
                nc.sync.dma_start(h1t[:, k, :], h1[k * 128 : (k + 1) * 128, :])
                nc.sync.dma_start(sct[:, k : k + 1], sc[k * 128 : (k + 1) * 128, :])

            ident_f = cpool.tile([128, 128], f32)
            make_identity(nc, ident_f[:])
            ident_r = cpool.tile([128, 128], xdt)
            nc.vector.tensor_copy(ident_r[:], ident_f[:])
            ones128x16 = cpool.tile([128, 16], f32)
            nc.vector.memset(ones128x16[:], 1.0)
            rs128 = cpool.tile([128, 1], f32)
            nc.vector.memset(rs128[:], 0.0)

            # ---- per-row bias: biasT[p, m, r] = (prep_r @ wp + child_r @ wc)[m*128+p]
            pc_prep = cpool.tile([128, KD, R], bdt)
            pc_child = cpool.tile([128, KD, R], bdt)
            for r in range(R):
                for k in range(KD):
                    src_p = xs[r, T - 2, k * 128 : (k + 1) * 128].unsqueeze(-1)
                    src_c = xs[r, T - 1, k * 128 : (k + 1) * 128].unsqueeze(-1)
                    if not X_BF16:
                        src_p = src_p.bitcast(bdt)
                        src_c = src_c.bitcast(bdt)
                    nc.sync.dma_start(pc_prep[:, k, r : r + 1], src_p)
                    nc.sync.dma_start(pc_child[:, k, r : r + 1], src_c)
            biasT = cpool.tile([128, KP, R], f32)
            for m in range(KP):
                bps = mmp_pool.tile([128, R], f32, tag="mm")
                for k in range(KD):
                    nc.tensor.matmul(
                        bps[:],
                        wpt[:, k, m * 128 : (m + 1) * 128],
                        pc_prep[:, k, :],
                        start=(k == 0),
                        stop=False,
                    )
                for k in range(KD):
                    nc.tensor.matmul(
                        bps[:],
                        wct[:, k, m * 128 : (m + 1) * 128],
                        pc_child[:, k, :],
                        start=False,
                        stop=(k == KD - 1),
                    )
                nc.vector.tensor_copy(biasT[:, m, :], bps[:])

            # ---- main loop ----
            for r in range(R):
                if SAFE_SCORER:
                    s_sb = tail_pool.tile([128, J16], f32, tag="ssb")
                    sc_ps = None
                else:
                    sc_ps = scp_pool.tile([128, J16], f32, tag="scps")
                for c in range(CH):
                    xn = xn_pool.tile([128, 4, D], xdt, tag="xn")
                    for jj in range(4):
                        t0 = c * NTOK + jj * 128
                        nc.sync.dma_start(xn[:, jj, :], xs[r, t0 : t0 + 128, :])
                    # transpose x to [d, tok]
                    xts = []
                    for k in range(KD):
                        xp = xtp_pool.tile([128, NTOK], xdt, tag="xtps")
                        for jj in range(4):
                            nc.tensor.transpose(
                                xp[:, jj * 128 : (jj + 1) * 128],
                                xn[:, jj, k * 128 : (k + 1) * 128],
                                ident_r[:],
                            )
                        xt = xt_pool.tile([128, NTOK], xdt, tag="xt")
                        nc.vector.tensor_copy(xt[:], xp[:])
                        xts.append(xt)
                    # layer 1: y1 = tanh(W1.T @ xT + bias)
                    y1s = []
                    for m in range(KP):
                        ps = mmp_pool.tile([128, NTOK], f32, tag="mm")
                        for k in range(KD):
                            nc.tensor.matmul(
                                ps[:],
                                w1t[:, k, m * 128 : (m + 1) * 128],
                                xts[k][:],
                                start=(k == 0),
                                stop=(k == KD - 1),
                            )
                        y1 = y_pool.tile([128, NTOK], f32r, tag="y1")
                        nc.scalar.activation(
                            y1[:], ps[:], AF.Tanh, bias=biasT[:, m, r : r + 1]
                        )
                        y1s.append(y1)
                    # layer 2
                    y2s = []
                    for m in range(KP):
                        ps = mmp_pool.tile([128, NTOK], f32, tag="mm")
                        for k in range(KP):
                            nc.tensor.matmul(
                                ps[:],
                                h0t[:, k, m * 128 : (m + 1) * 128],
                                y1s[k][:],
                                start=(k == 0),
                                stop=(k == KP - 1),
                            )
                        y2 = y_pool.tile([128, NTOK], f32r, tag="y2")
                        nc.scalar.activation(y2[:], ps[:], AF.Tanh)
                        y2s.append(y2)
                    # layer 3
                    y3s = []
                    for m in range(KP):
                        ps = mmp_pool.tile([128, NTOK], f32, tag="mm")
                        for k in range(KP):
                            nc.tensor.matmul(
                                ps[:],
                                h1t[:, k, m * 128 : (m + 1) * 128],
                                y2s[k][:],
                                start=(k == 0),
                                stop=(k == KP - 1),
                            )
                        y3 = y_pool.tile([128, NTOK], f32, tag="y3")
                        nc.scalar.activation(y3[:], ps[:], AF.Tanh)
                        y3s.append(y3)
                    # scorer: scores land [tok-on-partitions]
                    for jj in range(4):
                        col = c * 4 + jj
                        if SAFE_SCORER:
                            s1 = mmp_pool.tile([128, 1], f32, tag="mm")
                            for k in range(KP):
                                nc.tensor.matmul(
                                    s1[:],
                                    y3s[k][:, jj * 128 : (jj + 1) * 128],
                                    sct[:, k : k + 1],
                                    start=(k == 0),
                                    stop=(k == KP - 1),
                                )
                            nc.vector.tensor_copy(s_sb[:, col : col + 1], s1[:])
                        else:
                            for k in range(KP):
                                nc.tensor.matmul(
                                    sc_ps[:, col : col + 1],
                                    y3s[k][:, jj * 128 : (jj + 1) * 128],
                                    sct[:, k : k + 1],
                                    start=(k == 0),
                                    stop=(k == KP - 1),
                                )
                # ---- tail: masked softmax over the row ----
                # exp into cols 0:16 of a 128-wide pad tile; full-width PE
                # transpose; only rows 0:16 of the result are read.
                e_pad = tail_pool.tile([128, 128], f32, tag="esb")
                nc.scalar.activation(e_pad[:, 0:J16], s_sb[:] if SAFE_SCORER else sc_ps[:], AF.Exp)
                et_ps = tlp_pool.tile([128, 128], f32, tag="tl")
                nc.tensor.transpose(et_ps[:], e_pad[:], ident_f[:])
                mku8 = tail_pool.tile([16, 128], u8, tag="mku8")
                nc.sync.dma_start(
                    mku8[:], mk[r, 0:2048].rearrange("(j p) -> j p", p=128)
                )
                mf = tail_pool.tile([16, 128], f32, tag="mf")
                nc.vector.tensor_copy(mf[:], mku8[:])
                me = tail_pool.tile([16, 128], f32, tag="me")
                nc.vector.tensor_tensor(
                    out=me[:], in0=et_ps[0:16, :], in1=mf[:], op=ALU.mult
                )
                rs = tail_pool.tile([16, 1], f32, tag="rs")
                nc.vector.reduce_sum(rs[:], me[:], axis=mybir.AxisListType.X)
                nc.vector.tensor_copy(rs128[0:16, :], rs[:])
                rb_ps = tlp_pool.tile([16, 1], f32, tag="tl")
                nc.tensor.matmul(rb_ps[:], ones128x16[:], rs128[:])
                rb = tail_pool.tile([16, 1], f32, tag="rb")
                nc.vector.tensor_scalar_add(rb[:], rb_ps[:], 1e-7)
                rcp = tail_pool.tile([16, 1], f32, tag="rcp")
                nc.vector.reciprocal(rcp[:], rb[:])
                ot = tail_pool.tile([16, 128], f32, tag="ot")
                nc.vector.tensor_scalar_mul(ot[:], me[:], rcp[:])
                nc.sync.dma_start(
                    out[r, 0:1920].rearrange("(j p) -> j p", p=128), ot[0:15, :]
                )
                nc.sync.dma_start(
                    out[r, 1920:2046].rearrange("(j p) -> j p", p=126),
                    ot[15:16, 0:126],
                )
    nc.compile()
    return nc


def _get_nc():
    if "nc" not in _CACHE:
        _CACHE["nc"] = _build()
    return _CACHE["nc"]


def _make_in_maps(inputs):
    import ml_dtypes

    xdt = ml_dtypes.bfloat16 if X_BF16 else np.float32
    x = np.ascontiguousarray(np.asarray(inputs["x"], dtype=np.float32).astype(xdt))
    w1 = np.ascontiguousarray(np.asarray(inputs["proj_head"], dtype=np.float32).astype(xdt))
    wp = np.ascontiguousarray(np.asarray(inputs["proj_prep"], dtype=np.float32).astype(xdt))
    wc = np.ascontiguousarray(np.asarray(inputs["proj_child"], dtype=np.float32).astype(xdt))
    hw = np.asarray(inputs["hidden_W"], dtype=np.float32)
    sc = np.ascontiguousarray(np.asarray(inputs["scorer"], dtype=np.float32))
    mk = np.asarray(inputs["mask"]).astype(np.uint8).copy()
    mk[:, TH:] = 0  # prep/child rows are never head candidates
    in_maps = []
    for i in range(NCORES):
        in_maps.append(
            {
                "xs": np.ascontiguousarray(x[i * R : (i + 1) * R]),
                "w1": w1,
                "wp": wp,
                "wc": wc,
                "h0": np.ascontiguousarray(hw[0]),
                "h1": np.ascontiguousarray(hw[1]),
                "sc": sc,
                "mk": np.ascontiguousarray(mk[i * R : (i + 1) * R]),
            }
        )
    return in_maps


def _run(inputs, **kwargs):
    from concourse.bass_utils import run_bass_kernel_spmd

    nc = _get_nc()
    res = run_bass_kernel_spmd(
        nc, _make_in_maps(inputs), core_ids=list(range(NCORES)), **kwargs
    )
    out = np.concatenate([res.results[i]["out"] for i in range(NCORES)], axis=0)
    return out, res


def kernel(**inputs) -> np.ndarray:
    out, _ = _run(inputs)
    return out



# revision 5
# speedup vs baseline: 1.4093x; 1.4093x over previous
"""Trainium2 Bass kernel for nn_AttachmentPredictor.

Computation (per batch row b):
  head = x[b, :-2, :] @ proj_head + x[b,-2,:] @ proj_prep + x[b,-1,:] @ proj_child
  composed = tanh(head)                      # [T-2, P]
  composed = tanh(composed @ hidden_W[0])
  composed = tanh(composed @ hidden_W[1])
  scores = composed @ scorer                 # [T-2]
  out = where(mask, exp(scores), 0); out /= (sum(out) + 1e-7)

Sharding: pure data parallel, batch 64 -> 8 rows per core on 8 cores.

Layout: all activations transposed [P on partitions, tokens on free dim].
x is shipped bf16 and transposed HBM->SBUF by the DMA xbar engine
(dma_start_transpose), so the tensor engine only runs the real GEMMs.
All GEMMs run in bf16 (full-rate).  The per-row prep/child bias columns are
read from the transposed x tile (columns T-2, T-1) and accumulated on the PE,
then applied through the activation bias port.  The 2046 head tokens ride
along as 2048 (prep/child rows processed as garbage and masked out).

Emission is software-pipelined: chunks are processed in pairs so each
layer's tanh latency is hidden behind the sibling chunk's matmuls, the next
row's x transposes and bias are issued a row ahead, and the per-row masked
softmax tail is interleaved into the next row's main loop.
"""

import sys

import numpy as np

sys.path.insert(0, "/opt/trn_rl_repo")

B = 64
T = 2048
TH = 2046  # head tokens
D = 1024
P = 512
NCORES = 8
R = B // NCORES  # 8 batch rows per core
KD = D // 128  # 8 contraction chunks for layer 1
KP = P // 128  # 4 contraction chunks for layers 2/3/scorer
NTOK = 512  # tokens per chunk
CH = T // NTOK  # 4 chunks per row
J16 = T // 128  # 16 score sub-chunks of 128 tokens per row

_CACHE = {}


def _build(R=R):
    import concourse.bass as bass
    import concourse.mybir as mybir
    import concourse.tile as tile
    from concourse import bacc
    from concourse.masks import make_identity

    f32 = mybir.dt.float32
    bf16 = mybir.dt.bfloat16
    u8 = mybir.dt.uint8
    AF = mybir.ActivationFunctionType
    ALU = mybir.AluOpType

    nc = bacc.Bacc(
        "TRN2", target_bir_lowering=False, debug=False, num_devices=NCORES
    )

    xs = nc.dram_tensor("xs", [R, T, D], bf16, kind="ExternalInput").ap()
    w1 = nc.dram_tensor("w1", [D, P], bf16, kind="ExternalInput").ap()
    wp = nc.dram_tensor("wp", [D, P], bf16, kind="ExternalInput").ap()
    wc = nc.dram_tensor("wc", [D, P], bf16, kind="ExternalInput").ap()
    h0 = nc.dram_tensor("h0", [P, P], bf16, kind="ExternalInput").ap()
    h1 = nc.dram_tensor("h1", [P, P], bf16, kind="ExternalInput").ap()
    sc = nc.dram_tensor("sc", [P, 1], bf16, kind="ExternalInput").ap()
    mk = nc.dram_tensor("mk", [R, T], u8, kind="ExternalInput").ap()
    out = nc.dram_tensor("out", [R, TH], f32, kind="ExternalOutput").ap()

    with tile.TileContext(nc) as tc:
        with (
            tc.tile_pool(name="mmp_pool", bufs=5, space="PSUM") as mmp_pool,
            tc.tile_pool(name="scp_pool", bufs=1, space="PSUM") as scp_pool,
            tc.tile_pool(name="tlp_pool", bufs=2, space="PSUM") as tlp_pool,
            tc.tile_pool(name="wpool", bufs=1) as wpool,
            tc.tile_pool(name="cpool", bufs=1) as cpool,
            tc.tile_pool(name="xt_pool", bufs=2) as xt_pool,
            tc.tile_pool(name="y_pool", bufs=2 * KP) as y_pool,
            tc.tile_pool(name="tail_pool", bufs=2) as tail_pool,
        ):
            # ---- transposed x, one tile per row: xt[p, k, t] = x[t, k*128+p]
            xts = {}

            def issue_xt(r):
                xt = xt_pool.tile([128, KD, T], bf16, tag="xtr", name=f"xt{r}")
                xts[r] = xt
                for k in range(KD):
                    nc.sync.dma_start_transpose(
                        xt[:, k, :], xs[r, :, k * 128 : (k + 1) * 128]
                    )

            def issue_xt_chunk(r, c):
                t0 = c * NTOK
                for k in range(KD):
                    nc.sync.dma_start_transpose(
                        xts[r][:, k, t0 : t0 + NTOK],
                        xs[r, t0 : t0 + NTOK, k * 128 : (k + 1) * 128],
                    )

            # ---- prologue: row-0 transposes interleaved with weight loads so
            # the PE's first layer-1 group and the row-0 bias unblock early.
            xts[0] = xt_pool.tile([128, KD, T], bf16, tag="xtr", name="xt0")
            issue_xt_chunk(0, 0)
            w1t = wpool.tile([128, KD, P], bf16)
            nc.sync.dma_start(w1t[:], w1.rearrange("(k p) q -> p k q", p=128))
            issue_xt_chunk(0, 1)
            issue_xt_chunk(0, 3)
            wpt = wpool.tile([128, KD, P], bf16)
            nc.sync.dma_start(wpt[:], wp.rearrange("(k p) q -> p k q", p=128))
            wct = wpool.tile([128, KD, P], bf16)
            nc.sync.dma_start(wct[:], wc.rearrange("(k p) q -> p k q", p=128))
            h0t = wpool.tile([128, KP, P], bf16)
            nc.sync.dma_start(h0t[:], h0.rearrange("(k p) q -> p k q", p=128))
            h1t = wpool.tile([128, KP, P], bf16)
            nc.sync.dma_start(h1t[:], h1.rearrange("(k p) q -> p k q", p=128))
            issue_xt_chunk(0, 2)
            sct = wpool.tile([128, KP], bf16)
            nc.sync.dma_start(
                sct[:].unsqueeze(-1), sc.rearrange("(k p) s -> p k s", p=128)
            )
            mka = wpool.tile([16, R, 128], u8)
            nc.sync.dma_start(mka[:], mk.rearrange("r (j p) -> j r p", p=128))

            ident_f = cpool.tile([128, 128], f32)
            make_identity(nc, ident_f[:])
            ones128x16 = cpool.tile([128, 16], f32)
            nc.vector.memset(ones128x16[:], 1.0)
            rs128 = cpool.tile([128, 1], f32)
            nc.vector.memset(rs128[:], 0.0)
            biasT = cpool.tile([128, KP, R], f32)

            # ---- helpers -------------------------------------------------
            def emit_bias(r):
                """biasT[:,m,r] = wp.T @ x[r,T-2] + wc.T @ x[r,T-1] from the
                transposed-x columns T-2 / T-1."""
                for m in range(KP):
                    bp = mmp_pool.tile([128, 1], f32, tag="mm", name=f"bp{r}{m}")
                    mb = slice(m * 128, (m + 1) * 128)
                    for k in range(KD):
                        nc.tensor.matmul(
                            bp[:],
                            wpt[:, k, mb],
                            xts[r][:, k, T - 2 : T - 1],
                            start=(k == 0),
                            stop=False,
                        )
                    for k in range(KD):
                        nc.tensor.matmul(
                            bp[:],
                            wct[:, k, mb],
                            xts[r][:, k, T - 1 : T],
                            start=False,
                            stop=(k == KD - 1),
                        )
                    nc.vector.tensor_copy(biasT[:, m, r : r + 1], bp[:])

            def emit_l1(r, c, ys):
                t0 = c * NTOK
                for m in range(KP):
                    ps = mmp_pool.tile([128, NTOK], f32, tag="mm", name="l1ps")
                    mb = slice(m * 128, (m + 1) * 128)
                    for k in range(KD):
                        nc.tensor.matmul(
                            ps[:],
                            w1t[:, k, mb],
                            xts[r][:, k, t0 : t0 + NTOK],
                            start=(k == 0),
                            stop=(k == KD - 1),
                        )
                    y = y_pool.tile([128, NTOK], bf16, tag="y1", name="y1")
                    nc.scalar.activation(
                        y[:], ps[:], AF.Tanh, bias=biasT[:, m, r : r + 1]
                    )
                    ys[(c, m)] = y

            def emit_mid(wt, yin, c, ys, ytag):
                for m in range(KP):
                    ps = mmp_pool.tile([128, NTOK], f32, tag="mm", name="lps")
                    mb = slice(m * 128, (m + 1) * 128)
                    for k in range(KP):
                        nc.tensor.matmul(
                            ps[:],
                            wt[:, k, mb],
                            yin[(c, k)][:],
                            start=(k == 0),
                            stop=(k == KP - 1),
                        )
                    y = y_pool.tile([128, NTOK], bf16, tag=ytag, name=ytag)
                    nc.scalar.activation(y[:], ps[:], AF.Tanh)
                    ys[(c, m)] = y

            def emit_score(sc_ps, y3s, c):
                for jj in range(4):
                    col = c * 4 + jj
                    jb = slice(jj * 128, (jj + 1) * 128)
                    for k in range(KP):
                        nc.tensor.matmul(
                            sc_ps[:, col : col + 1],
                            y3s[(c, k)][:, jb],
                            sct[:, k : k + 1],
                            start=(k == 0),
                            stop=(k == KP - 1),
                        )

            # ---- per-row masked-softmax tail, emitted in pieces that are
            # interleaved into the NEXT row's main loop so the PE never waits
            # on the exp/mask/reduce chain.
            tails = {}

            def tail_a(r):
                st = tails[r]
                e_pad = tail_pool.tile([128, 128], f32, tag="esb", name="e_pad")
                nc.scalar.activation(e_pad[:, 0:J16], st["sc_ps"][:], AF.Exp)
                st["e_pad"] = e_pad

            def tail_b(r):
                st = tails[r]
                et_ps = tlp_pool.tile([128, 128], f32, tag="tl", name="et_ps")
                nc.tensor.transpose(et_ps[:], st["e_pad"][:], ident_f[:])
                st["et_ps"] = et_ps

            def tail_c1(r):
                st = tails[r]
                mf = tail_pool.tile([16, 128], f32, tag="mf", name="mf")
                nc.vector.tensor_copy(mf[:], mka[:, r, :])
                me = tail_pool.tile([16, 128], f32, tag="me", name="me")
                nc.vector.tensor_tensor(
                    out=me[:], in0=st["et_ps"][0:16, :], in1=mf[:], op=ALU.mult
                )
                rs = tail_pool.tile([16, 1], f32, tag="rs", name="rs")
                nc.vector.reduce_sum(rs[:], me[:], axis=mybir.AxisListType.X)
                nc.vector.tensor_copy(rs128[0:16, :], rs[:])
                st["me"] = me

            def tail_c2(r):
                st = tails[r]
                rb_ps = tlp_pool.tile([16, 1], f32, tag="tl", name="rb_ps")
                nc.tensor.matmul(rb_ps[:], ones128x16[:], rs128[:])
                st["rb_ps"] = rb_ps

            def tail_d(r):
                st = tails[r]
                rb = tail_pool.tile([16, 1], f32, tag="rb", name="rb")
                nc.vector.tensor_scalar_add(rb[:], st["rb_ps"][:], 1e-7)
                rcp = tail_pool.tile([16, 1], f32, tag="rcp", name="rcp")
                nc.vector.reciprocal(rcp[:], rb[:])
                ot = tail_pool.tile([16, 128], f32, tag="ot", name="ot")
                nc.vector.tensor_scalar_mul(ot[:], st["me"][:], rcp[:])
                nc.sync.dma_start(
                    out[r, 0:1920].rearrange("(j p) -> j p", p=128), ot[0:15, :]
                )
                nc.sync.dma_start(
                    out[r, 1920:2046].rearrange("(j p) -> j p", p=126),
                    ot[15:16, 0:126],
                )
                del tails[r]

            # row-0 bias must precede row-0's layer-1 activations in program
            # order (the tile deps don't reorder); later rows are emitted a
            # row ahead inside the loop.
            emit_bias(0)

            # ---- main loop: chunk pairs hide tanh latency; tail of row r-1
            # rides inside row r's first pair.
            for r in range(R):
                if r + 1 < R:
                    issue_xt(r + 1)
                if r > 0:
                    tail_a(r - 1)
                sc_ps = scp_pool.tile([128, J16], f32, tag="scps", name="sc_ps")
                tails[r] = {"sc_ps": sc_ps}
                for half in range(2):
                    ca, cb = 2 * half, 2 * half + 1
                    y1s, y2s, y3s = {}, {}, {}
                    emit_l1(r, ca, y1s)
                    emit_l1(r, cb, y1s)
                    if half == 0:
                        if r > 0:
                            tail_b(r - 1)
                    else:
                        if r + 1 < R:
                            emit_bias(r + 1)
                    emit_mid(h0t, y1s, ca, y2s, "y2")
                    emit_mid(h0t, y1s, cb, y2s, "y2")
                    if half == 0 and r > 0:
                        tail_c1(r - 1)
                    emit_mid(h1t, y2s, ca, y3s, "y3")
                    emit_mid(h1t, y2s, cb, y3s, "y3")
                    if half == 0 and r > 0:
                        tail_c2(r - 1)
                    emit_score(sc_ps, y3s, ca)
                    emit_score(sc_ps, y3s, cb)
                    if half == 0 and r > 0:
                        tail_d(r - 1)

            # final row's tail
            tail_a(R - 1)
            tail_b(R - 1)
            tail_c1(R - 1)
            tail_c2(R - 1)
            tail_d(R - 1)
    nc.compile()
    return nc


def _get_nc():
    if "nc" not in _CACHE:
        _CACHE["nc"] = _build()
    return _CACHE["nc"]


def _make_in_maps(inputs):
    import ml_dtypes

    bf = ml_dtypes.bfloat16
    x = np.ascontiguousarray(np.asarray(inputs["x"], dtype=np.float32).astype(bf))
    w1 = np.ascontiguousarray(np.asarray(inputs["proj_head"], dtype=np.float32).astype(bf))
    wp = np.ascontiguousarray(np.asarray(inputs["proj_prep"], dtype=np.float32).astype(bf))
    wc = np.ascontiguousarray(np.asarray(inputs["proj_child"], dtype=np.float32).astype(bf))
    hw = np.asarray(inputs["hidden_W"], dtype=np.float32).astype(bf)
    sc = np.ascontiguousarray(np.asarray(inputs["scorer"], dtype=np.float32).astype(bf))
    mk = np.asarray(inputs["mask"]).astype(np.uint8).copy()
    mk[:, TH:] = 0  # prep/child rows are never head candidates
    in_maps = []
    for i in range(NCORES):
        in_maps.append(
            {
                "xs": np.ascontiguousarray(x[i * R : (i + 1) * R]),
                "w1": w1,
                "wp": wp,
                "wc": wc,
                "h0": np.ascontiguousarray(hw[0]),
                "h1": np.ascontiguousarray(hw[1]),
                "sc": sc,
                "mk": np.ascontiguousarray(mk[i * R : (i + 1) * R]),
            }
        )
    return in_maps


def _run(inputs, **kwargs):
    from concourse.bass_utils import run_bass_kernel_spmd

    nc = _get_nc()
    res = run_bass_kernel_spmd(
        nc, _make_in_maps(inputs), core_ids=list(range(NCORES)), **kwargs
    )
    out = np.concatenate([res.results[i]["out"] for i in range(NCORES)], axis=0)
    return out, res


def kernel(**inputs) -> np.ndarray:
    out, _ = _run(inputs)
    return out


# revision 41
# speedup vs baseline: 2.3602x; 1.6748x over previous
"""Trainium2 Bass kernel for nn_AttachmentPredictor.

Computation (per batch row b):
  head = x[b, :-2, :] @ proj_head + x[b,-2,:] @ proj_prep + x[b,-1,:] @ proj_child
  composed = tanh(head)                      # [T-2, P]
  composed = tanh(composed @ hidden_W[0])
  composed = tanh(composed @ hidden_W[1])
  scores = composed @ scorer                 # [T-2]
  out = where(mask, exp(scores), 0); out /= (sum(out) + 1e-7)

Sharding: pure data parallel, batch 64 -> 8 rows per core on 8 cores.

Key algorithmic point: masked-out tokens contribute exactly zero to the
output (their exp(score) is multiplied by 0 and they are excluded from the
softmax sum), so only masked-in tokens are computed.  The host gathers each
row's masked-in tokens into a compact [PADT] layout (PADT = max row count
rounded up to 128, typically ~56% of T), the device runs the dense pipeline
on the compacted tokens, and the host scatters results back.  The compact
mask (1 for i < count, zero-padded to 16 blocks) drives the same masked
softmax tail as the dense kernel, so padding lanes vanish exactly.

Device layout: all activations transposed [P on partitions, tokens free].
x is shipped bf16 and transposed HBM->SBUF by the DMA xbar engine
(dma_start_transpose) - one 3D-AP DMA covers all 8 contraction blocks of a
token span.  All GEMMs run in bf16 (full rate).  Compact rows are laid out
[prep, child, 14 pad, tokens...], so the per-row prep/child bias columns
are simply columns 0/1 of the transposed tile; the bias is accumulated on
the PE and applied through the activation bias port.

Emission is software-pipelined: chunks are processed in pairs so tanh
latency hides behind the sibling chunk's matmuls; the next row's x
transposes and bias are issued a row ahead; the per-row masked-softmax tail
is interleaved into the next row's trailing chunk group.
"""

import sys

import numpy as np

sys.path.insert(0, "/opt/trn_rl_repo")

B = 64
T = 2048
TH = 2046  # head tokens
D = 1024
P = 512
NCORES = 8
R = B // NCORES  # 8 batch rows per core
KD = D // 128  # 8 contraction chunks for layer 1
KP = P // 128  # 4 contraction chunks for layers 2/3/scorer
FRONT = 16  # prep, child, 14 pad tokens at the head of each compact row
J16 = 16  # score blocks in the (zero-padded) tail

_CACHE = {}


def _chunks(PADT):
    """Token-chunk lengths covering PADT (PSUM limits a chunk to 512)."""
    chs = [512] * (PADT // 512)
    if PADT % 512:
        chs.append(PADT % 512)
    return chs


def _build(PADT):
    import concourse.bass as bass
    import concourse.mybir as mybir
    import concourse.tile as tile
    from concourse import bacc
    from concourse.masks import make_identity
    from concourse.tile_rust import add_dep_helper

    f32 = mybir.dt.float32
    bf16 = mybir.dt.bfloat16
    u8 = mybir.dt.uint8
    AF = mybir.ActivationFunctionType
    ALU = mybir.AluOpType

    TC = FRONT + PADT  # tokens per compact row on device
    CHS = _chunks(PADT)
    NCH = len(CHS)
    off = [FRONT + 512 * c for c in range(NCH)]
    # chunk groups: pairs hide tanh latency; an odd trailing chunk rides solo
    groups = [tuple(g for g in (2 * i, 2 * i + 1) if g < NCH)
              for i in range((NCH + 1) // 2)]

    nc = bacc.Bacc(
        "TRN2", target_bir_lowering=False, debug=False, num_devices=NCORES
    )

    xs = nc.dram_tensor("xs", [R, TC, D], bf16, kind="ExternalInput").ap()
    w1 = nc.dram_tensor("w1", [D, P], bf16, kind="ExternalInput").ap()
    wp = nc.dram_tensor("wp", [D, P], bf16, kind="ExternalInput").ap()
    wc = nc.dram_tensor("wc", [D, P], bf16, kind="ExternalInput").ap()
    h0 = nc.dram_tensor("h0", [P, P], bf16, kind="ExternalInput").ap()
    h1 = nc.dram_tensor("h1", [P, P], bf16, kind="ExternalInput").ap()
    sc = nc.dram_tensor("sc", [P, 1], bf16, kind="ExternalInput").ap()
    mk = nc.dram_tensor("mk", [R, J16 * 128], u8, kind="ExternalInput").ap()
    out = nc.dram_tensor("out", [R, J16 * 128], f32, kind="ExternalOutput").ap()

    with tile.TileContext(nc) as tc:
        with (
            tc.tile_pool(name="mmp_pool", bufs=5, space="PSUM") as mmp_pool,
            tc.tile_pool(name="scp_pool", bufs=1, space="PSUM") as scp_pool,
            tc.tile_pool(name="tlp_pool", bufs=2, space="PSUM") as tlp_pool,
            tc.tile_pool(name="wpool", bufs=1) as wpool,
            tc.tile_pool(name="cpool", bufs=1) as cpool,
            tc.tile_pool(name="xt_pool", bufs=2) as xt_pool,
            tc.tile_pool(name="y_pool", bufs=2 * KP) as y_pool,
            tc.tile_pool(name="tail_pool", bufs=2) as tail_pool,
        ):
            # ---- transposed x, one tile per row: xt[p, k, t] = x[t, k*128+p]
            #
            # InstDmaTransposeAnt is INVISIBLE to the tile dependency tracker
            # (its ISA-lowered access patterns aren't mapped back to tile
            # regions), so every data edge touching these writes is added
            # explicitly with add_dep_helper: readers wait for the covering
            # transposes (RAW), and a row's transposes wait for the last
            # reader of the ring slot they recycle (WAR).  All transposes
            # stay on the SP queue: cross-queue DMA waits lower to the wrong
            # DMA-completion semaphore.
            xts = {}
            xt_wr = {}  # r -> list of (lo, hi, mybir inst) transpose writes
            last_rd = {}  # r -> last emitted matmul reading xts[r]

            def issue_xt_span(r, lo, hi):
                # One DMA transposes [hi-lo, D] -> [128, KD, hi-lo] via a 3D
                # out AP.  Span bounds must be 16-aligned (xbar tile rows)
                # and < TC so the (k, t) out dims can't merge to 2D.
                assert lo % 16 == 0 and hi % 16 == 0 and hi - lo < TC
                bi = nc.sync.dma_start_transpose(
                    xts[r][:, :, lo:hi], xs[r, lo:hi, :]
                )
                xt_wr.setdefault(r, []).append((lo, hi, bi.ins))
                if r - 2 in last_rd:
                    add_dep_helper(bi.ins, last_rd[r - 2], reason="xt WAR")

            MID = (TC // 2 // 16) * 16

            def issue_xt(r):
                xts[r] = xt_pool.tile(
                    [128, KD, TC], bf16, tag="xtr", name=f"xt{r}"
                )
                issue_xt_span(r, 0, MID)
                issue_xt_span(r, MID, TC)

            def dep_on_xt(mm, r, lo, hi):
                for wlo, whi, di in xt_wr[r]:
                    if wlo < hi and lo < whi:
                        add_dep_helper(mm.ins, di, reason="xt RAW")

            # ---- prologue.  Each DMA holds the SP SEQ until the serialized
            # HWDGE accepts it (~1.2us per DMA), so row 0 is transposed in
            # chunk-sized spans ordered so each input lands just before its
            # consumer: w1+span0 for the first layer-1 group (span0 also
            # carries the prep/child bias columns), wp/wc for the bias,
            # h0/h1 for layers 2/3, trailing spans last.
            xts[0] = xt_pool.tile([128, KD, TC], bf16, tag="xtr", name="xt0")
            w1t = wpool.tile([128, KD, P], bf16)
            wpt = wpool.tile([128, KD, P], bf16)
            wct = wpool.tile([128, KD, P], bf16)
            h0t = wpool.tile([128, KP, P], bf16)
            h1t = wpool.tile([128, KP, P], bf16)
            sct = wpool.tile([128, KP], bf16)
            mka = wpool.tile([J16, R, 128], u8)

            r0_spans = [(off[c], off[c] + CHS[c]) for c in range(NCH)]
            r0_spans[0] = (0, r0_spans[0][1])  # include the FRONT columns

            nc.sync.dma_start(w1t[:], w1.rearrange("(k p) q -> p k q", p=128))
            issue_xt_span(0, *r0_spans[0])
            nc.sync.dma_start(wpt[:], wp.rearrange("(k p) q -> p k q", p=128))
            nc.sync.dma_start(wct[:], wc.rearrange("(k p) q -> p k q", p=128))
            if NCH > 1:
                issue_xt_span(0, *r0_spans[1])
            nc.sync.dma_start(h0t[:], h0.rearrange("(k p) q -> p k q", p=128))
            nc.sync.dma_start(h1t[:], h1.rearrange("(k p) q -> p k q", p=128))
            for s in r0_spans[2:]:
                issue_xt_span(0, *s)
            nc.sync.dma_start(
                sct[:].unsqueeze(-1), sc.rearrange("(k p) s -> p k s", p=128)
            )
            nc.sync.dma_start(mka[:], mk.rearrange("r (j p) -> j r p", p=128))

            ident_f = cpool.tile([128, 128], f32)
            make_identity(nc, ident_f[:])
            ones128x16 = cpool.tile([128, 16], f32)
            nc.vector.memset(ones128x16[:], 1.0)
            rs128 = cpool.tile([128, 1], f32)
            nc.vector.memset(rs128[:], 0.0)
            biasT = cpool.tile([128, KP, R], f32)

            # PE warm-up: the tensor engine only reaches full clock after
            # ~3us of continuous execution; burn the ramp on dummy identity
            # matmuls during the DMA-bound prologue.
            for i in range(14):
                dmy = tlp_pool.tile([128, 128], f32, tag="tl", name=f"wm{i}")
                nc.tensor.matmul(dmy[:], ident_f[:], ident_f[:])

            # ---- helpers -------------------------------------------------
            def emit_bias(r):
                """biasT[:,m,r] = wp.T @ prep + wc.T @ child from compact
                columns 0/1.  Small PSUM tiles on the 'tl' ring so the bias
                never recycles (waits on) the layer-matmul ring."""
                for m in range(KP):
                    bp = tlp_pool.tile([128, 1], f32, tag="tl", name=f"bp{r}{m}")
                    mb = slice(m * 128, (m + 1) * 128)
                    for k in range(KD):
                        mm = nc.tensor.matmul(
                            bp[:],
                            wpt[:, k, mb],
                            xts[r][:, k, 0:1],
                            start=(k == 0),
                            stop=False,
                        )
                        if m == 0 and k == 0:
                            dep_on_xt(mm, r, 0, 2)
                        last_rd[r] = mm.ins
                    for k in range(KD):
                        mm = nc.tensor.matmul(
                            bp[:],
                            wct[:, k, mb],
                            xts[r][:, k, 1:2],
                            start=False,
                            stop=(k == KD - 1),
                        )
                        last_rd[r] = mm.ins
                    nc.vector.tensor_copy(biasT[:, m, r : r + 1], bp[:])

            def emit_l1_mm(r, c, m):
                t0, L = off[c], CHS[c]
                ps = mmp_pool.tile([128, L], f32, tag="mm", name="l1ps")
                mb = slice(m * 128, (m + 1) * 128)
                for k in range(KD):
                    mm = nc.tensor.matmul(
                        ps[:],
                        w1t[:, k, mb],
                        xts[r][:, k, t0 : t0 + L],
                        start=(k == 0),
                        stop=(k == KD - 1),
                    )
                    if m == 0 and k == 0:
                        dep_on_xt(mm, r, t0, t0 + L)
                    last_rd[r] = mm.ins
                return ps

            def emit_l1_tanh(r, c, m, ps, ys):
                y = y_pool.tile([128, CHS[c]], bf16, tag="y1", name="y1")
                nc.scalar.activation(
                    y[:], ps[:], AF.Tanh, bias=biasT[:, m, r : r + 1]
                )
                ys[(c, m)] = y

            def emit_l1(r, c, ys):
                for m in range(KP):
                    ps = emit_l1_mm(r, c, m)
                    emit_l1_tanh(r, c, m, ps, ys)

            def emit_mid(wt, yin, c, ys, ytag):
                for m in range(KP):
                    ps = mmp_pool.tile([128, CHS[c]], f32, tag="mm", name="lps")
                    mb = slice(m * 128, (m + 1) * 128)
                    for k in range(KP):
                        nc.tensor.matmul(
                            ps[:],
                            wt[:, k, mb],
                            yin[(c, k)][:],
                            start=(k == 0),
                            stop=(k == KP - 1),
                        )
                    y = y_pool.tile([128, CHS[c]], bf16, tag=ytag, name=ytag)
                    nc.scalar.activation(y[:], ps[:], AF.Tanh)
                    ys[(c, m)] = y

            def emit_score(sc_ps, y3s, c):
                for jj in range(CHS[c] // 128):
                    col = 4 * c + jj
                    jb = slice(jj * 128, (jj + 1) * 128)
                    for k in range(KP):
                        nc.tensor.matmul(
                            sc_ps[:, col : col + 1],
                            y3s[(c, k)][:, jb],
                            sct[:, k : k + 1],
                            start=(k == 0),
                            stop=(k == KP - 1),
                        )

            # ---- per-row masked-softmax tail, emitted in pieces that are
            # interleaved into the NEXT row's trailing chunk group (where
            # they double as latency fillers for the unpaired chunk).
            tails = {}

            def tail_a(r):
                st = tails[r]
                e_pad = tail_pool.tile([128, 128], f32, tag="esb", name="e_pad")
                nc.scalar.activation(e_pad[:, 0:J16], st["sc_ps"][:], AF.Exp)
                st["e_pad"] = e_pad

            def tail_b(r):
                st = tails[r]
                et_ps = tlp_pool.tile([128, 128], f32, tag="tl", name="et_ps")
                nc.tensor.transpose(et_ps[:], st["e_pad"][:], ident_f[:])
                st["et_ps"] = et_ps

            def tail_c1(r):
                st = tails[r]
                mf = tail_pool.tile([16, 128], f32, tag="mf", name="mf")
                nc.vector.tensor_copy(mf[:], mka[:, r, :])
                me = tail_pool.tile([16, 128], f32, tag="me", name="me")
                nc.vector.tensor_tensor(
                    out=me[:], in0=st["et_ps"][0:16, :], in1=mf[:], op=ALU.mult
                )
                rs = tail_pool.tile([16, 1], f32, tag="rs", name="rs")
                nc.vector.reduce_sum(rs[:], me[:], axis=mybir.AxisListType.X)
                nc.vector.tensor_copy(rs128[0:16, :], rs[:])
                st["me"] = me

            def tail_c2(r):
                st = tails[r]
                rb_ps = tlp_pool.tile([16, 1], f32, tag="tl", name="rb_ps")
                nc.tensor.matmul(rb_ps[:], ones128x16[:], rs128[:])
                st["rb_ps"] = rb_ps

            def tail_d(r):
                st = tails[r]
                rb = tail_pool.tile([16, 1], f32, tag="rb", name="rb")
                nc.vector.tensor_scalar_add(rb[:], st["rb_ps"][:], 1e-7)
                rcp = tail_pool.tile([16, 1], f32, tag="rcp", name="rcp")
                nc.vector.reciprocal(rcp[:], rb[:])
                ot = tail_pool.tile([16, 128], f32, tag="ot", name="ot")
                nc.vector.tensor_scalar_mul(ot[:], st["me"][:], rcp[:])
                nc.sync.dma_start(
                    out[r, :].rearrange("(j p) -> j p", p=128), ot[:]
                )
                del tails[r]

            # ---- main loop ----------------------------------------------
            for r in range(R):
                if r + 1 < R:
                    issue_xt(r + 1)
                if r > 0:
                    tail_a(r - 1)
                sc_ps = scp_pool.tile([128, J16], f32, tag="scps", name="sc_ps")
                tails[r] = {"sc_ps": sc_ps}
                for gi, grp in enumerate(groups):
                    first, last = gi == 0, gi == len(groups) - 1
                    y1s, y2s, y3s = {}, {}, {}
                    if first and r == 0:
                        # row 0: run c0's layer-1 matmuls before the bias so
                        # the PE isn't head-of-line blocked on wp/wc; the c0
                        # tanhs (which need the bias) follow.
                        pss = [emit_l1_mm(0, grp[0], m) for m in range(KP)]
                        emit_bias(0)
                        for m in range(KP):
                            emit_l1_tanh(0, grp[0], m, pss[m], y1s)
                        for c in grp[1:]:
                            emit_l1(r, c, y1s)
                    else:
                        for c in grp:
                            emit_l1(r, c, y1s)
                    if last and r + 1 < R:
                        emit_bias(r + 1)
                    for c in grp:
                        emit_mid(h0t, y1s, c, y2s, "y2")
                    if last and r > 0:
                        tail_b(r - 1)
                        tail_c1(r - 1)
                    for c in grp:
                        emit_mid(h1t, y2s, c, y3s, "y3")
                    if last and r > 0:
                        tail_c2(r - 1)
                    for c in grp:
                        emit_score(sc_ps, y3s, c)
                    if last and r > 0:
                        tail_d(r - 1)

            # final row's tail
            tail_a(R - 1)
            tail_b(R - 1)
            tail_c1(R - 1)
            tail_c2(R - 1)
            tail_d(R - 1)
    nc.compile()
    return nc


def _get_nc(PADT=None):
    if PADT is None:
        PADT = _CACHE.get("last_padt", 1152)
    _CACHE["last_padt"] = PADT
    key = ("nc", PADT)
    if key not in _CACHE:
        _CACHE[key] = _build(PADT)
    return _CACHE[key]


def _prep(inputs):
    """Compact the masked-in tokens per row; returns (in_maps, gidx, cnt,
    PADT)."""
    import ml_dtypes

    bf = ml_dtypes.bfloat16
    x = np.asarray(inputs["x"], dtype=np.float32)
    mask = np.asarray(inputs["mask"]).astype(bool)
    head_mask = mask[:, :TH]
    gidx = [np.nonzero(head_mask[b])[0] for b in range(B)]
    cnt = np.array([len(g) for g in gidx])
    PADT = max(128, int(np.ceil(max(cnt.max(), 1) / 128)) * 128)
    TC = FRONT + PADT

    xc = np.zeros((B, TC, D), dtype=bf)
    for b in range(B):
        xc[b, 0] = x[b, T - 2].astype(bf)
        xc[b, 1] = x[b, T - 1].astype(bf)
        xc[b, FRONT : FRONT + cnt[b]] = x[b, gidx[b]].astype(bf)
    mkc = np.zeros((B, J16 * 128), dtype=np.uint8)
    for b in range(B):
        mkc[b, : cnt[b]] = 1

    w1 = np.ascontiguousarray(np.asarray(inputs["proj_head"], dtype=np.float32).astype(bf))
    wpw = np.ascontiguousarray(np.asarray(inputs["proj_prep"], dtype=np.float32).astype(bf))
    wcw = np.ascontiguousarray(np.asarray(inputs["proj_child"], dtype=np.float32).astype(bf))
    hw = np.asarray(inputs["hidden_W"], dtype=np.float32).astype(bf)
    scw = np.ascontiguousarray(np.asarray(inputs["scorer"], dtype=np.float32).astype(bf))

    in_maps = []
    for i in range(NCORES):
        in_maps.append(
            {
                "xs": np.ascontiguousarray(xc[i * R : (i + 1) * R]),
                "w1": w1,
                "wp": wpw,
                "wc": wcw,
                "h0": np.ascontiguousarray(hw[0]),
                "h1": np.ascontiguousarray(hw[1]),
                "sc": scw,
                "mk": np.ascontiguousarray(mkc[i * R : (i + 1) * R]),
            }
        )
    return in_maps, gidx, cnt, PADT


def _run(inputs, **kwargs):
    from concourse.bass_utils import run_bass_kernel_spmd

    in_maps, gidx, cnt, PADT = _prep(inputs)
    nc = _get_nc(PADT)
    res = run_bass_kernel_spmd(
        nc, in_maps, core_ids=list(range(NCORES)), **kwargs
    )
    oc = np.concatenate([res.results[i]["out"] for i in range(NCORES)], axis=0)
    full = np.zeros((B, TH), dtype=np.float32)
    for b in range(B):
        full[b, gidx[b]] = oc[b, : cnt[b]]
    return full, res


def kernel(**inputs) -> np.ndarray:
    out, _ = _run(inputs)
    return out
